# revision 19
# baseline (speedup 1.0000x reference)
"""EVA-02 ViT attention block (LoRA + rope + rel-pos-bias) on 8 TRN2 NeuronCores.

Data-parallel over batch (8 images per core). Per core:
  - LoRA merged into qkv weights on the host; q-scale and v-bias folded away.
  - q/k projected in transposed layout (channels on partitions), v natural,
    fp32r matmuls at full PE rate (free dim >= 256 via image pairs).
  - rope in transposed layout; the pair swap is a DVE stream_shuffle (channels
    host-permuted so rope pairs sit 16 apart within 32-partition blocks).
  - scores transposed (S^T[j,i]); rel-pos bias added via an identity matmul
    into the same PSUM accumulation group; exp on ScalarE without max
    subtraction (scores are O(1)); probs bf16.
  - attn@v with v stationary emits O^T directly; softmax denominators via
    ones-vector matmuls; 1/x as exp(-ln x) on ScalarE; normalization applied
    in the O^T eviction multiply.
  - output projection back to natural [n, c] layout; bias via K=1 ones matmul.
"""
import numpy as np
import ml_dtypes

B, N, C, H, R = 64, 197, 768, 12, 24
D = C // H               # 64
NCORES = 8
BPC = B // NCORES        # images per core
F2 = 2 * N               # 394
N0, N1 = 128, N - 128    # token chunks: 128 + 69

_cache = {}

SHUF_MASK = list(range(16, 32)) + list(range(0, 16))


def _perm64():
    p = []
    for blk in range(2):
        base = blk * 32
        p += [base + 2 * t for t in range(16)]
        p += [base + 2 * t + 1 for t in range(16)]
    return np.array(p)


def _swap16_rows(a):
    out = np.empty_like(a)
    for s in range(a.shape[0] // 32):
        out[s * 32:s * 32 + 16] = a[s * 32 + 16:s * 32 + 32]
        out[s * 32 + 16:s * 32 + 32] = a[s * 32:s * 32 + 16]
    return out


def build_program(n_pairs=BPC // 2, use_shuffle=True):
    import concourse.bass as bass
    import concourse.tile as tile
    from concourse import bacc, mybir

    f32, f32r, bf16 = mybir.dt.float32, mybir.dt.float32r, mybir.dt.bfloat16
    AF = mybir.ActivationFunctionType
    OP = mybir.AluOpType

    nc = bacc.Bacc("TRN2", target_bir_lowering=False, debug=False)
    n_img = 2 * n_pairs

    xt_d = nc.dram_tensor("xt", [n_pairs, C, F2], f32, kind="ExternalInput")
    wt_d = nc.dram_tensor("wt", [C, 3 * C], f32, kind="ExternalInput")
    bq_d = nc.dram_tensor("bq", [128, 6], f32, kind="ExternalInput")
    bqs_d = nc.dram_tensor("bqs", [128, 6], f32, kind="ExternalInput")
    cs_d = nc.dram_tensor("cs", [2, 128, F2], f32, kind="ExternalInput")
    rpb_d = nc.dram_tensor("rpbt", [H, 2, 128, N], bf16, kind="ExternalInput")
    ident_d = nc.dram_tensor("ident", [128, 128], bf16, kind="ExternalInput")
    projt_d = nc.dram_tensor("projt", [C, C], f32, kind="ExternalInput")
    projb_d = nc.dram_tensor("projb", [1, C], f32, kind="ExternalInput")
    onesr_d = nc.dram_tensor("onesr", [1, 128], f32, kind="ExternalInput")
    y_d = nc.dram_tensor("y", [n_img, N, C], f32, kind="ExternalOutput")

    from contextlib import ExitStack
    with tile.TileContext(nc) as tc:
        with ExitStack() as stk:
            pool = lambda name, bufs, **kw: stk.enter_context(
                tc.tile_pool(name=name, bufs=bufs, **kw))
            # NOTE: bufs is per-tag. PSUM budget: qkps 2 + vps 1 + psA 1 +
            # psB 1 + aops 1 + sums 1 + yps 1 = 8 banks exactly.
            constp = pool("const", 1)
            xtp = pool("xt", 2)
            qkps = pool("qkps", 2, space="PSUM")
            vps = pool("vps", 1, space="PSUM")
            qkbfp = pool("qkbf", 3)
            ropep = pool("rope", 2)
            ropet = pool("ropet", 3)
            vsbp = pool("vsb", 5)
            scps = pool("scps", 1, space="PSUM")
            probsp = pool("probs", 13)
            aops = pool("aops", 1, space="PSUM")
            aosbp = pool("aosb", 7)
            sumsp = pool("sums", 1, space="PSUM")
            rsbp = pool("rsb", 2)
            rbc = pool("rbc", 2)
            yps = pool("yps", 1, space="PSUM")
            ysbp = pool("ysb", 2)
            otp = pool("otp", 8)
            # ---- constants ----
            wt_sb = []
            for cc in range(6):
                t = constp.tile([128, 3 * C], f32r, tag=f"wt{cc}")
                nc.sync.dma_start(t[:], wt_d[cc * 128:(cc + 1) * 128, :].bitcast(f32r))
                wt_sb.append(t)
            projt_sb = []
            for cc in range(6):
                t = constp.tile([128, C], f32r, tag=f"pt{cc}")
                nc.sync.dma_start(t[:], projt_d[cc * 128:(cc + 1) * 128, :].bitcast(f32r))
                projt_sb.append(t)
            projb_sb = constp.tile([1, C], f32r, tag="pb")
            nc.sync.dma_start(projb_sb[:], projb_d[:].bitcast(f32r))
            bq_sb = constp.tile([128, 6], f32, tag="bq")
            nc.sync.dma_start(bq_sb[:], bq_d[:])
            bqs_sb = constp.tile([128, 6], f32, tag="bqs")
            nc.sync.dma_start(bqs_sb[:], bqs_d[:])
            cos_sb = constp.tile([128, F2], f32, tag="cos")
            nc.sync.dma_start(cos_sb[:], cs_d[0])
            spm_sb = constp.tile([128, F2], f32, tag="spm")
            nc.sync.dma_start(spm_sb[:], cs_d[1])
            rpb_sb = []
            for h in range(H):
                a = constp.tile([128, N], bf16, tag=f"rpb{h}a")
                b = constp.tile([128, N], bf16, tag=f"rpb{h}b")
                nc.sync.dma_start(a[:], rpb_d[h, 0])
                nc.sync.dma_start(b[:], rpb_d[h, 1])
                rpb_sb.append((a, b))
            ident_sb = constp.tile([128, 128], bf16, tag="ident")
            nc.sync.dma_start(ident_sb[:], ident_d[:])
            ones_row = constp.tile([1, 128], f32r, tag="onesr")
            nc.sync.dma_start(ones_row[:], onesr_d[:].bitcast(f32r))
            # E-band: column 11 is ones; slicing [:, 11-h:23-h] gives a
            # [128, 12] selector with ones in column h.
            eband = constp.tile([128, 23], bf16, tag="eband")
            nc.vector.memset(eband[:], 0.0)
            nc.vector.memset(eband[:, 11:12], 1.0)

            for p in range(n_pairs):
                # ---- load xT for this image pair ----
                xt_sb = []
                for cc in range(6):
                    t = xtp.tile([128, F2], f32r, tag=f"xt{cc}")
                    nc.sync.dma_start(t[:], xt_d[p, cc * 128:(cc + 1) * 128, :].bitcast(f32r))
                    xt_sb.append(t)

                # ---- q/k projection (transposed out) + rope ----
                rope_out = {}
                for m in range(12):
                    ps = qkps.tile([128, F2], f32, tag="qkps", padded_shape=[128, 512])
                    for cc in range(6):
                        nc.tensor.matmul(
                            ps[:],
                            lhsT=wt_sb[cc][:, m * 128:(m + 1) * 128],
                            rhs=xt_sb[cc][:],
                            start=(cc == 0), stop=(cc == 5))
                    src = qkbfp.tile([128, F2], bf16, tag="qkbf")
                    nc.scalar.activation(src[:], ps[:], AF.Copy)

                    qs = ropet.tile([128, F2], bf16, tag="qs")
                    if use_shuffle:
                        nc.vector.stream_shuffle(qs[:], src[:], SHUF_MASK)
                    else:
                        s4 = src.rearrange("(a b c) f -> a b c f", b=2, c=16)
                        q4 = qs.rearrange("(a b c) f -> a b c f", b=2, c=16)
                        nc.sync.dma_start(q4[:, 0], s4[:, 1])
                        nc.sync.dma_start(q4[:, 1], s4[:, 0])
                    u = ropet.tile([128, F2], bf16, tag="u")
                    v = ropet.tile([128, F2], bf16, tag="v")
                    if m < 6:
                        nc.vector.scalar_tensor_tensor(
                            out=u[:], in0=src[:], scalar=bq_sb[:, m:m + 1],
                            in1=cos_sb[:], op0=OP.add, op1=OP.mult)
                        nc.vector.scalar_tensor_tensor(
                            out=v[:], in0=qs[:], scalar=bqs_sb[:, m:m + 1],
                            in1=spm_sb[:], op0=OP.add, op1=OP.mult)
                    else:
                        nc.vector.tensor_mul(u[:], src[:], cos_sb[:])
                        nc.vector.tensor_mul(v[:], qs[:], spm_sb[:])
                    ro = ropep.tile([128, F2], bf16, tag=f"ro{m}")
                    nc.vector.tensor_add(ro[:], u[:], v[:])
                    rope_out[m] = ro

                # ---- v projection (natural out) ----
                v_sb = []
                for ic in range(2):
                    vts = [vsbp.tile([128, C], bf16, tag="vsb", name=f"vsb{ic}{i}")
                           for i in range(2)]
                    for nck, (n_off, n_sz) in enumerate(((0, N0), (N0, N1))):
                        for ch in range(2):
                            ps = vps.tile([128, 384], f32, tag="vps", padded_shape=[128, 512])
                            for cc in range(6):
                                nc.tensor.matmul(
                                    ps[0:n_sz, :],
                                    lhsT=xt_sb[cc][:, ic * N + n_off:ic * N + n_off + n_sz],
                                    rhs=wt_sb[cc][:, 2 * C + ch * 384:2 * C + (ch + 1) * 384],
                                    start=(cc == 0), stop=(cc == 5))
                            nc.scalar.activation(
                                vts[nck][0:n_sz, ch * 384:(ch + 1) * 384],
                                ps[0:n_sz, :], AF.Copy)
                    v_sb.append(vts)

                # ---- attention (transposed), hp-major ----
                probs_all = {}
                ao_sb = []
                for hp in range(6):
                    qro = rope_out[hp]
                    kro = rope_out[hp + 6]
                    ao = aops.tile([128, F2], f32, tag="aops", padded_shape=[128, 512])
                    for ic in range(2):
                        qoff = ic * N
                        psA = scps.tile([128, F2], f32, tag="psA", padded_shape=[128, 512])
                        psB = scps.tile([128, F2], f32, tag="psB", padded_shape=[128, 512])
                        for ph in range(2):
                            h = 2 * hp + ph
                            cr = ph * N
                            qv = qro[ph * 64:(ph + 1) * 64, qoff:qoff + N]
                            nc.tensor.matmul(
                                psA[:, cr:cr + N], lhsT=ident_sb[:],
                                rhs=rpb_sb[h][0][:], start=True, stop=False)
                            nc.tensor.matmul(
                                psA[:, cr:cr + N],
                                lhsT=kro[ph * 64:(ph + 1) * 64, qoff:qoff + 128],
                                rhs=qv, start=False, stop=True)
                            nc.tensor.matmul(
                                psB[0:N1, cr:cr + N], lhsT=ident_sb[0:N1, 0:N1],
                                rhs=rpb_sb[h][1][0:N1, :], start=True, stop=False)
                            nc.tensor.matmul(
                                psB[0:N1, cr:cr + N],
                                lhsT=kro[ph * 64:(ph + 1) * 64, qoff + 128:qoff + N],
                                rhs=qv, start=False, stop=True)
                        prA = probsp.tile([128, F2], bf16, tag="prA")
                        prB = probsp.tile([128, F2], bf16, tag="prB")
                        nc.scalar.activation(prA[:], psA[:], AF.Exp)
                        nc.scalar.activation(prB[0:N1, :], psB[0:N1, :], AF.Exp)
                        probs_all[(hp, ic)] = (prA, prB)
                        for ph in range(2):
                            h = 2 * hp + ph
                            cr = ph * N
                            nc.tensor.matmul(
                                ao[ph * 64:(ph + 1) * 64, ic * N:(ic + 1) * N],
                                lhsT=v_sb[ic][0][:, h * 64:(h + 1) * 64],
                                rhs=prA[:, cr:cr + N], start=True, stop=False)
                            nc.tensor.matmul(
                                ao[ph * 64:(ph + 1) * 64, ic * N:(ic + 1) * N],
                                lhsT=v_sb[ic][1][0:N1, h * 64:(h + 1) * 64],
                                rhs=prB[0:N1, cr:cr + N], start=False, stop=True)

                    aot = aosbp.tile([128, F2], f32, tag="aosb")
                    nc.scalar.activation(aot[:], ao[:], AF.Copy)
                    ao_sb.append(aot)

                # ---- softmax denominators, batched per image ----
                sums_ps = sumsp.tile([12, F2], f32, tag="sums", padded_shape=[12, 512])
                for ic in range(2):
                    for hp in range(6):
                        prA, prB = probs_all[(hp, ic)]
                        for ph in range(2):
                            h = 2 * hp + ph
                            cr = ph * N
                            first = (hp == 0 and ph == 0)
                            last = (hp == 5 and ph == 1)
                            nc.tensor.matmul(
                                sums_ps[:, ic * N:(ic + 1) * N],
                                lhsT=eband[:, 11 - h:23 - h],
                                rhs=prA[:, cr:cr + N],
                                start=first, stop=False, skip_group_check=True)
                            nc.tensor.matmul(
                                sums_ps[:, ic * N:(ic + 1) * N],
                                lhsT=eband[0:N1, 11 - h:23 - h],
                                rhs=prB[0:N1, cr:cr + N],
                                start=False, stop=last, skip_group_check=True)

                # ---- normalization: r = exp(-ln(sums)) ----
                lnt = rsbp.tile([12, F2], f32, tag="lnt")
                nc.scalar.activation(lnt[:], sums_ps[:], AF.Ln)
                rsm = rsbp.tile([12, F2], f32, tag="rsm")
                nc.scalar.activation(rsm[:], lnt[:], AF.Exp, scale=-1.0)
                ot_sb = []
                for hp in range(6):
                    # broadcast r rows across partitions via 0-stride DMA
                    rb = rbc.tile([128, F2], f32, tag="rbc")
                    nc.sync.dma_start(
                        rb[0:64, :],
                        rsm[2 * hp:2 * hp + 1, :].unsqueeze(1)
                        .broadcast_to((1, 64, F2)))
                    nc.sync.dma_start(
                        rb[64:128, :],
                        rsm[2 * hp + 1:2 * hp + 2, :].unsqueeze(1)
                        .broadcast_to((1, 64, F2)))
                    ot = otp.tile([128, F2], f32r, tag="ot")
                    nc.vector.tensor_mul(ot[:], ao_sb[hp][:], rb[:])
                    ot_sb.append(ot)

                # ---- output projection ----
                for ic in range(2):
                    img = 2 * p + ic
                    for n_off, n_sz in ((0, N0), (N0, N1)):
                        yt = ysbp.tile([128, C], f32, tag="ysb")
                        for ch in range(2):
                            ps = yps.tile([128, 384], f32, tag="yps", padded_shape=[128, 512])
                            for cc in range(6):
                                nc.tensor.matmul(
                                    ps[0:n_sz, :],
                                    lhsT=ot_sb[cc][:, ic * N + n_off:ic * N + n_off + n_sz],
                                    rhs=projt_sb[cc][:, ch * 384:(ch + 1) * 384],
                                    start=(cc == 0), stop=False)
                            nc.tensor.matmul(
                                ps[0:n_sz, :],
                                lhsT=ones_row[:, 0:n_sz],
                                rhs=projb_sb[:, ch * 384:(ch + 1) * 384],
                                start=False, stop=True)
                            nc.scalar.activation(
                                yt[0:n_sz, ch * 384:(ch + 1) * 384],
                                ps[0:n_sz, :], AF.Copy)
                        nc.sync.dma_start(
                            y_d[img, n_off:n_off + n_sz, :], yt[0:n_sz, :])
    nc.compile()
    return nc


def host_prepare(inputs):
    x = np.asarray(inputs["x"], np.float32)
    qkv_w = np.asarray(inputs["qkv_w"], np.float32)
    scale = D ** -0.5
    Wq = qkv_w[:C] + np.asarray(inputs["lora_q_b"]) @ np.asarray(inputs["lora_q_a"])
    Wk = qkv_w[C:2 * C] + np.asarray(inputs["lora_k_b"]) @ np.asarray(inputs["lora_k_a"])
    Wv = qkv_w[2 * C:] + np.asarray(inputs["lora_v_b"]) @ np.asarray(inputs["lora_v_a"])
    p64 = _perm64()
    perm = (np.arange(H)[:, None] * D + p64[None, :]).ravel()
    Wq_de = (Wq * scale)[perm]
    bq_de = (np.asarray(inputs["q_bias"], np.float32) * scale)[perm]
    Wk_de = Wk[perm]
    wt = np.ascontiguousarray(np.concatenate([Wq_de, Wk_de, Wv], 0).T)

    bq = np.ascontiguousarray(bq_de.reshape(6, 128).T)
    bqs = np.ascontiguousarray(
        np.stack([_swap16_rows(bq_de[i * 128:(i + 1) * 128]) for i in range(6)], 1))

    cos_f = np.ones((N, D), np.float32)
    cos_f[1:] = np.asarray(inputs["rope_cos"], np.float32)
    sin_f = np.zeros((N, D), np.float32)
    sin_f[1:] = np.asarray(inputs["rope_sin"], np.float32)
    cos_de = np.ascontiguousarray(cos_f[:, p64].T)
    spm = np.ascontiguousarray(sin_f[:, p64].T)
    for blk in range(2):
        spm[blk * 32:blk * 32 + 16] *= -1.0
    cs = np.stack([
        np.tile(np.vstack([cos_de, cos_de]), (1, 2)),
        np.tile(np.vstack([spm, spm]), (1, 2)),
    ]).astype(np.float32)

    rel_table = np.asarray(inputs["rel_table"], np.float32)
    rel_index = np.asarray(inputs["rel_index"])
    rpb = rel_table[rel_index.reshape(-1)].reshape(N, N, H)
    rpbT = rpb.transpose(2, 1, 0)  # [h, j, i]
    rpbt = np.zeros((H, 2, 128, N), ml_dtypes.bfloat16)
    rpbt[:, 0] = rpbT[:, 0:128, :].astype(ml_dtypes.bfloat16)
    rpbt[:, 1, 0:N1] = rpbT[:, 128:N, :].astype(ml_dtypes.bfloat16)

    ident = np.eye(128, dtype=ml_dtypes.bfloat16)
    proj_w = np.asarray(inputs["proj_w"], np.float32)
    projt = np.ascontiguousarray(proj_w.T)
    projb = (np.asarray(inputs["proj_b"], np.float32)
             + proj_w @ np.asarray(inputs["v_bias"], np.float32)).reshape(1, C)

    xt = x.transpose(0, 2, 1)  # [B, C, N]
    xt_pairs = np.ascontiguousarray(
        xt.reshape(B // 2, 2, C, N).transpose(0, 2, 1, 3).reshape(B // 2, C, 2 * N))

    shared = dict(wt=wt, bq=bq, bqs=bqs, cs=cs, rpbt=rpbt, ident=ident,
                  projt=projt, projb=projb,
                  onesr=np.ones((1, 128), np.float32))
    per_core = []
    ppc = BPC // 2
    for c in range(NCORES):
        m = dict(shared)
        m["xt"] = np.ascontiguousarray(xt_pairs[c * ppc:(c + 1) * ppc])
        per_core.append(m)
    return per_core


def kernel(**inputs):
    from concourse.bass_utils import run_bass_kernel_spmd
    in_maps = host_prepare(inputs)
    if "nc" not in _cache:
        _cache["nc"] = build_program()
    nc = _cache["nc"]
    res = run_bass_kernel_spmd(nc, in_maps, list(range(NCORES))).results
    y = np.concatenate([res[c]["y"] for c in range(NCORES)], 0)
    return np.ascontiguousarray(y.astype(np.float32))


# revision 21
# speedup vs baseline: 5800.3264x; 5800.3264x over previous
"""EVA-02 ViT attention block (LoRA + rope + rel-pos-bias) on 8 TRN2 NeuronCores.

Data-parallel over batch (8 images per core). Per core:
  - LoRA merged into qkv weights on the host; q-scale and v-bias folded away.
  - q/k projected in transposed layout (channels on partitions), v natural,
    fp32r matmuls at full PE rate (free dim >= 256 via image pairs).
  - rope in transposed layout; the pair swap is a DVE stream_shuffle (channels
    host-permuted so rope pairs sit 16 apart within 32-partition blocks).
  - scores transposed (S^T[j,i]); rel-pos bias added via an identity matmul
    into the same PSUM accumulation group; exp on ScalarE without max
    subtraction (scores are O(1)); probs bf16.
  - attn@v with v stationary emits O^T directly; softmax denominators via
    ones-vector matmuls; 1/x as exp(-ln x) on ScalarE; normalization applied
    in the O^T eviction multiply.
  - output projection back to natural [n, c] layout; bias via K=1 ones matmul.
"""
import numpy as np
import ml_dtypes

B, N, C, H, R = 64, 197, 768, 12, 24
D = C // H               # 64
NCORES = 8
BPC = B // NCORES        # images per core
F2 = 2 * N               # 394
F4 = 4 * N               # 788
N0, N1 = 128, N - 128    # token chunks: 128 + 69

_cache = {}

SHUF_MASK = list(range(16, 32)) + list(range(0, 16))


def _perm64():
    p = []
    for blk in range(2):
        base = blk * 32
        p += [base + 2 * t for t in range(16)]
        p += [base + 2 * t + 1 for t in range(16)]
    return np.array(p)


def _swap16_rows(a):
    out = np.empty_like(a)
    for s in range(a.shape[0] // 32):
        out[s * 32:s * 32 + 16] = a[s * 32 + 16:s * 32 + 32]
        out[s * 32 + 16:s * 32 + 32] = a[s * 32:s * 32 + 16]
    return out


def build_program(n_pairs=BPC // 2, use_shuffle=True, repeat=1):
    import concourse.bass as bass
    import concourse.tile as tile
    from concourse import bacc, mybir

    f32, f32r, bf16 = mybir.dt.float32, mybir.dt.float32r, mybir.dt.bfloat16
    AF = mybir.ActivationFunctionType
    OP = mybir.AluOpType

    nc = bacc.Bacc("TRN2", target_bir_lowering=False, debug=False)
    n_img = 2 * n_pairs

    xt_d = nc.dram_tensor("xt", [n_pairs, C, F2], f32, kind="ExternalInput")
    wt_d = nc.dram_tensor("wt", [C, 3 * C], f32, kind="ExternalInput")
    bq_d = nc.dram_tensor("bq", [128, 6], f32, kind="ExternalInput")
    bqs_d = nc.dram_tensor("bqs", [128, 6], f32, kind="ExternalInput")
    cs_d = nc.dram_tensor("cs", [2, 128, F4], f32, kind="ExternalInput")
    rpb_d = nc.dram_tensor("rpbt", [H, 2, 128, N], bf16, kind="ExternalInput")
    ident_d = nc.dram_tensor("ident", [128, 128], bf16, kind="ExternalInput")
    projt_d = nc.dram_tensor("projt", [C, C], f32, kind="ExternalInput")
    projb_d = nc.dram_tensor("projb", [1, C], f32, kind="ExternalInput")
    y_d = nc.dram_tensor("y", [n_img, N, C], f32, kind="ExternalOutput")

    from contextlib import ExitStack
    with tile.TileContext(nc) as tc:
        with ExitStack() as stk:
            pool = lambda name, bufs, **kw: stk.enter_context(
                tc.tile_pool(name=name, bufs=bufs, **kw))
            # NOTE: bufs is per-tag. PSUM budget: qkps 2 + vps 1 + psA 1 +
            # psB 1 + aops 1 + sums 1 + yps 1 = 8 banks exactly.
            constp = pool("const", 1)
            xtp = pool("xt", 2)
            qkps = pool("qkps", 2, space="PSUM")
            vps = pool("vps", 1, space="PSUM")
            qkbfp = pool("qkbf", 2)
            ropep = pool("rope", 2)
            ropet = pool("ropet", 4)
            vsbp = pool("vsb", 10)
            scps = pool("scps", 1, space="PSUM")
            probsp = pool("probs", 13)
            aops = pool("aops", 1, space="PSUM")
            sumsp = pool("sums", 1, space="PSUM")
            rsbp = pool("rsb", 2)
            rbc = pool("rbc", 3)
            yps = pool("yps", 1, space="PSUM")
            ysbp = pool("ysb", 3)
            otp = pool("otp", 8)

            # ---- constants (batched DMAs) ----
            wt_all = constp.tile([128, 6 * 3 * C], f32r, tag="wtall")
            nc.sync.dma_start(
                wt_all[:],
                wt_d.rearrange("(cc p) j -> p (cc j)", cc=6).bitcast(f32r))
            wt_sb = [wt_all[:, cc * 3 * C:(cc + 1) * 3 * C] for cc in range(6)]
            pt_all = constp.tile([128, 6 * C], f32r, tag="ptall")
            nc.sync.dma_start(
                pt_all[:],
                projt_d.rearrange("(cc p) j -> p (cc j)", cc=6).bitcast(f32r))
            projt_sb = [pt_all[:, cc * C:(cc + 1) * C] for cc in range(6)]
            rpb_all = constp.tile([128, H * 2 * N], bf16, tag="rpball")
            nc.sync.dma_start(
                rpb_all[:], rpb_d.rearrange("h c p j -> p (h c j)"))
            rpb_sb = [(rpb_all[:, (h * 2) * N:(h * 2 + 1) * N],
                       rpb_all[:, (h * 2 + 1) * N:(h * 2 + 2) * N])
                      for h in range(H)]
            projb_bc = constp.tile([128, C], f32, tag="pbbc")
            nc.gpsimd.dma_start(
                projb_bc[:],
                projb_d[:].unsqueeze(1).broadcast_to((1, 128, C)))
            bq_sb = constp.tile([128, 6], f32, tag="bq")
            nc.sync.dma_start(bq_sb[:], bq_d[:])
            bqs_sb = constp.tile([128, 6], f32, tag="bqs")
            nc.sync.dma_start(bqs_sb[:], bqs_d[:])
            cos_sb = constp.tile([128, F4], f32, tag="cos")
            nc.sync.dma_start(cos_sb[:], cs_d[0])
            spm_sb = constp.tile([128, F4], f32, tag="spm")
            nc.sync.dma_start(spm_sb[:], cs_d[1])
            ident_sb = constp.tile([128, 128], bf16, tag="ident")
            nc.sync.dma_start(ident_sb[:], ident_d[:])
            # E-band: column 11 is ones; slicing [:, 11-h:23-h] gives a
            # [128, 12] selector with ones in column h.
            eband = constp.tile([128, 23], bf16, tag="eband")
            nc.vector.memset(eband[:], 0.0)
            nc.vector.memset(eband[:, 11:12], 1.0)

            qk_quad = {}
            v_pairs = {}

            def attention(p, par, xt_ref):
                """Scores/attn/normalize/proj for image pair p (quad slot par)."""
                v_sb = v_pairs.pop(p)
                probs_all = {}
                ao_list = []
                for hp in range(6):
                    qro = qk_quad[hp + 100]
                    kro = qk_quad[hp + 6 + 100]
                    ao = aops.tile([128, F2], f32, tag="aops",
                                   padded_shape=[128, 512], name=f"ao{p}{hp}")
                    for ic in range(2):
                        qoff = (par * 2 + ic) * N
                        psA = scps.tile([128, F2], f32, tag="psA",
                                        padded_shape=[128, 512], name=f"psA{p}{hp}{ic}")
                        psB = scps.tile([128, F2], f32, tag="psB",
                                        padded_shape=[128, 512], name=f"psB{p}{hp}{ic}")
                        for ph in range(2):
                            h = 2 * hp + ph
                            cr = ph * N
                            qv = qro[ph * 64:(ph + 1) * 64, qoff:qoff + N]
                            nc.tensor.matmul(
                                psA[:, cr:cr + N], lhsT=ident_sb[:],
                                rhs=rpb_sb[h][0], start=True, stop=False)
                            nc.tensor.matmul(
                                psA[:, cr:cr + N],
                                lhsT=kro[ph * 64:(ph + 1) * 64, qoff:qoff + 128],
                                rhs=qv, start=False, stop=True)
                            nc.tensor.matmul(
                                psB[0:N1, cr:cr + N], lhsT=ident_sb[0:N1, 0:N1],
                                rhs=rpb_sb[h][1][0:N1, :], start=True, stop=False)
                            nc.tensor.matmul(
                                psB[0:N1, cr:cr + N],
                                lhsT=kro[ph * 64:(ph + 1) * 64, qoff + 128:qoff + N],
                                rhs=qv, start=False, stop=True)
                        prA = probsp.tile([128, F2], bf16, tag="prA",
                                          name=f"prA{p}{hp}{ic}")
                        prB = probsp.tile([128, F2], bf16, tag="prB",
                                          name=f"prB{p}{hp}{ic}")
                        nc.scalar.activation(prA[:], psA[:], AF.Exp)
                        nc.scalar.activation(prB[0:N1, :], psB[0:N1, :], AF.Exp)
                        probs_all[(hp, ic)] = (prA, prB)
                        for ph in range(2):
                            h = 2 * hp + ph
                            cr = ph * N
                            nc.tensor.matmul(
                                ao[ph * 64:(ph + 1) * 64, ic * N:(ic + 1) * N],
                                lhsT=v_sb[ic][0][:, h * 64:(h + 1) * 64],
                                rhs=prA[:, cr:cr + N], start=True, stop=False)
                            nc.tensor.matmul(
                                ao[ph * 64:(ph + 1) * 64, ic * N:(ic + 1) * N],
                                lhsT=v_sb[ic][1][0:N1, h * 64:(h + 1) * 64],
                                rhs=prB[0:N1, cr:cr + N], start=False, stop=True)
                    ao_list.append(ao)

                # ---- softmax denominators, batched per image ----
                sums_ps = sumsp.tile([12, F2], f32, tag="sums",
                                     padded_shape=[12, 512], name=f"sums{p}")
                for ic in range(2):
                    for hp in range(6):
                        prA, prB = probs_all[(hp, ic)]
                        for ph in range(2):
                            h = 2 * hp + ph
                            cr = ph * N
                            first = (hp == 0 and ph == 0)
                            last = (hp == 5 and ph == 1)
                            nc.tensor.matmul(
                                sums_ps[:, ic * N:(ic + 1) * N],
                                lhsT=eband[:, 11 - h:23 - h],
                                rhs=prA[:, cr:cr + N],
                                start=first, stop=False, skip_group_check=True)
                            nc.tensor.matmul(
                                sums_ps[:, ic * N:(ic + 1) * N],
                                lhsT=eband[0:N1, 11 - h:23 - h],
                                rhs=prB[0:N1, cr:cr + N],
                                start=False, stop=last, skip_group_check=True)

                # ---- normalization: r = exp(-ln(sums)) ----
                lnt = rsbp.tile([12, F2], f32, tag="lnt", name=f"lnt{p}")
                nc.scalar.activation(lnt[:], sums_ps[:], AF.Ln)
                rsm = rsbp.tile([12, F2], f32, tag="rsm", name=f"rsm{p}")
                nc.scalar.activation(rsm[:], lnt[:], AF.Exp, scale=-1.0)
                ot_sb = []
                for hp in range(6):
                    # broadcast r rows across partitions via 0-stride DMA
                    rb = rbc.tile([128, F2], f32, tag="rbc", name=f"rb{p}{hp}")
                    nc.gpsimd.dma_start(
                        rb[0:64, :],
                        rsm[2 * hp:2 * hp + 1, :].unsqueeze(1)
                        .broadcast_to((1, 64, F2)))
                    nc.gpsimd.dma_start(
                        rb[64:128, :],
                        rsm[2 * hp + 1:2 * hp + 2, :].unsqueeze(1)
                        .broadcast_to((1, 64, F2)))
                    ot = otp.tile([128, F2], f32r, tag="ot", name=f"ot{p}{hp}")
                    nc.vector.tensor_mul(ot[:], ao_list[hp][:], rb[:])
                    ot_sb.append(ot)

                # ---- output projection (bias added at eviction) ----
                for ic in range(2):
                    img = (2 * p + ic) % n_img
                    for n_off, n_sz in ((0, N0), (N0, N1)):
                        yt = ysbp.tile([128, C], f32, tag="ysb",
                                       name=f"yt{p}{ic}{n_off}")
                        for ch in range(2):
                            ps = yps.tile([128, 384], f32, tag="yps",
                                          padded_shape=[128, 512],
                                          name=f"yps{p}{ic}{n_off}{ch}")
                            for cc in range(6):
                                nc.tensor.matmul(
                                    ps[0:n_sz, :],
                                    lhsT=ot_sb[cc][:, ic * N + n_off:ic * N + n_off + n_sz],
                                    rhs=projt_sb[cc][:, ch * 384:(ch + 1) * 384],
                                    start=(cc == 0), stop=(cc == 5))
                            nc.vector.tensor_add(
                                yt[0:n_sz, ch * 384:(ch + 1) * 384],
                                ps[0:n_sz, :],
                                projb_bc[0:n_sz, ch * 384:(ch + 1) * 384])
                        nc.sync.dma_start(
                            y_d[img, n_off:n_off + n_sz, :], yt[0:n_sz, :])

            total_pairs = repeat * n_pairs
            for pi in range(total_pairs):
                p = pi % n_pairs
                par = pi % 2
                # ---- load xT for this image pair ----
                xt_sb = []
                for cc in range(6):
                    t = xtp.tile([128, F2], f32r, tag=f"xt{cc}", name=f"xt{pi}{cc}")
                    nc.sync.dma_start(
                        t[:], xt_d[p, cc * 128:(cc + 1) * 128, :].bitcast(f32r))
                    xt_sb.append(t)

                # ---- q/k projection into quad tiles ----
                if par == 0:
                    for m in range(12):
                        qk_quad[m] = qkbfp.tile(
                            [128, F4], bf16, tag=f"qk{m}", name=f"qk{pi}{m}")
                for m in range(12):
                    ps = qkps.tile([128, F2], f32, tag="qkps",
                                   padded_shape=[128, 512], name=f"qkp{pi}{m}")
                    for cc in range(6):
                        nc.tensor.matmul(
                            ps[:],
                            lhsT=wt_sb[cc][:, m * 128:(m + 1) * 128],
                            rhs=xt_sb[cc][:],
                            start=(cc == 0), stop=(cc == 5))
                    dst = qk_quad[m][:, par * F2:(par + 1) * F2]
                    if m < 6:
                        nc.scalar.activation(dst, ps[:], AF.Copy)
                    else:
                        nc.vector.tensor_copy(dst, ps[:])

                # ---- v projection (natural out) ----
                v_sb = []
                for ic in range(2):
                    vts = [vsbp.tile([128, C], bf16, tag="vsb",
                                     name=f"vsb{pi}{ic}{i}") for i in range(2)]
                    for nck, (n_off, n_sz) in enumerate(((0, N0), (N0, N1))):
                        for ch in range(2):
                            ps = vps.tile([128, 384], f32, tag="vps",
                                          padded_shape=[128, 512],
                                          name=f"vps{pi}{ic}{nck}{ch}")
                            for cc in range(6):
                                nc.tensor.matmul(
                                    ps[0:n_sz, :],
                                    lhsT=xt_sb[cc][:, ic * N + n_off:ic * N + n_off + n_sz],
                                    rhs=wt_sb[cc][:, 2 * C + ch * 384:2 * C + (ch + 1) * 384],
                                    start=(cc == 0), stop=(cc == 5))
                            nc.scalar.activation(
                                vts[nck][0:n_sz, ch * 384:(ch + 1) * 384],
                                ps[0:n_sz, :], AF.Copy)
                    v_sb.append(vts)
                v_pairs[p] = v_sb

                # ---- rope on the full quad, then attention for both pairs ----
                if par == 1 or pi == total_pairs - 1:
                    fw = F4 if par == 1 else F2
                    for m in range(12):
                        src = qk_quad[m]
                        qs = ropet.tile([128, F4], bf16, tag="qs", name=f"qs{pi}{m}")
                        nc.vector.stream_shuffle(qs[:, 0:fw], src[:, 0:fw], SHUF_MASK)
                        u = ropet.tile([128, F4], bf16, tag="u", name=f"u{pi}{m}")
                        v = ropet.tile([128, F4], bf16, tag="v", name=f"v{pi}{m}")
                        if m < 6:
                            nc.vector.scalar_tensor_tensor(
                                out=u[:, 0:fw], in0=src[:, 0:fw],
                                scalar=bq_sb[:, m:m + 1],
                                in1=cos_sb[:, 0:fw], op0=OP.add, op1=OP.mult)
                            nc.vector.scalar_tensor_tensor(
                                out=v[:, 0:fw], in0=qs[:, 0:fw],
                                scalar=bqs_sb[:, m:m + 1],
                                in1=spm_sb[:, 0:fw], op0=OP.add, op1=OP.mult)
                        else:
                            nc.vector.tensor_mul(u[:, 0:fw], src[:, 0:fw],
                                                 cos_sb[:, 0:fw])
                            nc.vector.tensor_mul(v[:, 0:fw], qs[:, 0:fw],
                                                 spm_sb[:, 0:fw])
                        ro = ropep.tile([128, F4], bf16, tag=f"ro{m}",
                                        name=f"ro{pi}{m}")
                        nc.vector.tensor_add(ro[:, 0:fw], u[:, 0:fw], v[:, 0:fw])
                        qk_quad[m + 100] = ro
                    if par == 1:
                        attention((pi - 1) % n_pairs, 0, None)
                    attention(p, par, None)
    nc.compile()
    return nc


def host_prepare(inputs):
    x = np.asarray(inputs["x"], np.float32)
    qkv_w = np.asarray(inputs["qkv_w"], np.float32)
    scale = D ** -0.5
    Wq = qkv_w[:C] + np.asarray(inputs["lora_q_b"]) @ np.asarray(inputs["lora_q_a"])
    Wk = qkv_w[C:2 * C] + np.asarray(inputs["lora_k_b"]) @ np.asarray(inputs["lora_k_a"])
    Wv = qkv_w[2 * C:] + np.asarray(inputs["lora_v_b"]) @ np.asarray(inputs["lora_v_a"])
    p64 = _perm64()
    perm = (np.arange(H)[:, None] * D + p64[None, :]).ravel()
    Wq_de = (Wq * scale)[perm]
    bq_de = (np.asarray(inputs["q_bias"], np.float32) * scale)[perm]
    Wk_de = Wk[perm]
    wt = np.ascontiguousarray(np.concatenate([Wq_de, Wk_de, Wv], 0).T)

    bq = np.ascontiguousarray(bq_de.reshape(6, 128).T)
    bqs = np.ascontiguousarray(
        np.stack([_swap16_rows(bq_de[i * 128:(i + 1) * 128]) for i in range(6)], 1))

    cos_f = np.ones((N, D), np.float32)
    cos_f[1:] = np.asarray(inputs["rope_cos"], np.float32)
    sin_f = np.zeros((N, D), np.float32)
    sin_f[1:] = np.asarray(inputs["rope_sin"], np.float32)
    cos_de = np.ascontiguousarray(cos_f[:, p64].T)
    spm = np.ascontiguousarray(sin_f[:, p64].T)
    for blk in range(2):
        spm[blk * 32:blk * 32 + 16] *= -1.0
    cs = np.stack([
        np.tile(np.vstack([cos_de, cos_de]), (1, 4)),
        np.tile(np.vstack([spm, spm]), (1, 4)),
    ]).astype(np.float32)

    rel_table = np.asarray(inputs["rel_table"], np.float32)
    rel_index = np.asarray(inputs["rel_index"])
    rpb = rel_table[rel_index.reshape(-1)].reshape(N, N, H)
    rpbT = rpb.transpose(2, 1, 0)  # [h, j, i]
    rpbt = np.zeros((H, 2, 128, N), ml_dtypes.bfloat16)
    rpbt[:, 0] = rpbT[:, 0:128, :].astype(ml_dtypes.bfloat16)
    rpbt[:, 1, 0:N1] = rpbT[:, 128:N, :].astype(ml_dtypes.bfloat16)

    ident = np.eye(128, dtype=ml_dtypes.bfloat16)
    proj_w = np.asarray(inputs["proj_w"], np.float32)
    projt = np.ascontiguousarray(proj_w.T)
    projb = (np.asarray(inputs["proj_b"], np.float32)
             + proj_w @ np.asarray(inputs["v_bias"], np.float32)).reshape(1, C)

    xt = x.transpose(0, 2, 1)  # [B, C, N]
    xt_pairs = np.ascontiguousarray(
        xt.reshape(B // 2, 2, C, N).transpose(0, 2, 1, 3).reshape(B // 2, C, 2 * N))

    shared = dict(wt=wt, bq=bq, bqs=bqs, cs=cs, rpbt=rpbt, ident=ident,
                  projt=projt, projb=projb)
    per_core = []
    ppc = BPC // 2
    for c in range(NCORES):
        m = dict(shared)
        m["xt"] = np.ascontiguousarray(xt_pairs[c * ppc:(c + 1) * ppc])
        per_core.append(m)
    return per_core


def kernel(**inputs):
    from concourse.bass_utils import run_bass_kernel_spmd
    in_maps = host_prepare(inputs)
    if "nc" not in _cache:
        _cache["nc"] = build_program()
    nc = _cache["nc"]
    res = run_bass_kernel_spmd(nc, in_maps, list(range(NCORES))).results
    y = np.concatenate([res[c]["y"] for c in range(NCORES)], 0)
    return np.ascontiguousarray(y.astype(np.float32))


# revision 28
# speedup vs baseline: 5841.7290x; 1.0071x over previous
"""EVA-02 ViT attention block (LoRA + rope + rel-pos-bias) on 8 TRN2 NeuronCores.

Data-parallel over batch (8 images per core). Per core:
  - LoRA merged into qkv weights on the host; q-scale and v-bias folded away.
  - q/k projected in transposed layout (channels on partitions), v natural,
    fp32r matmuls at full PE rate (free dim >= 256 via image pairs).
  - rope in transposed layout; the pair swap is a DVE stream_shuffle (channels
    host-permuted so rope pairs sit 16 apart within 32-partition blocks).
  - scores transposed (S^T[j,i]); rel-pos bias added via an identity matmul
    into the same PSUM accumulation group; exp on ScalarE without max
    subtraction (scores are O(1)); probs bf16.
  - attn@v with v stationary emits O^T directly; softmax denominators via
    ones-vector matmuls; 1/x as exp(-ln x) on ScalarE; normalization applied
    in the O^T eviction multiply.
  - output projection back to natural [n, c] layout; bias via K=1 ones matmul.
"""
import numpy as np
import ml_dtypes

B, N, C, H, R = 64, 197, 768, 12, 24
D = C // H               # 64
NCORES = 8
BPC = B // NCORES        # images per core
F2 = 2 * N               # 394
F4 = 4 * N               # 788
N0, N1 = 128, N - 128    # token chunks: 128 + 69

_cache = {}

SHUF_MASK = list(range(16, 32)) + list(range(0, 16))


def _perm64():
    p = []
    for blk in range(2):
        base = blk * 32
        p += [base + 2 * t for t in range(16)]
        p += [base + 2 * t + 1 for t in range(16)]
    return np.array(p)


def _swap16_rows(a):
    out = np.empty_like(a)
    for s in range(a.shape[0] // 32):
        out[s * 32:s * 32 + 16] = a[s * 32 + 16:s * 32 + 32]
        out[s * 32 + 16:s * 32 + 32] = a[s * 32:s * 32 + 16]
    return out


def build_program(n_pairs=BPC // 2, use_shuffle=True, repeat=1):
    import concourse.bass as bass
    import concourse.tile as tile
    from concourse import bacc, mybir

    f32, f32r, bf16 = mybir.dt.float32, mybir.dt.float32r, mybir.dt.bfloat16
    AF = mybir.ActivationFunctionType
    OP = mybir.AluOpType

    nc = bacc.Bacc("TRN2", target_bir_lowering=False, debug=False)
    n_img = 2 * n_pairs

    xt_d = nc.dram_tensor("xt", [n_pairs, C, F2], f32, kind="ExternalInput")
    wt_d = nc.dram_tensor("wt", [C, 3 * C], f32, kind="ExternalInput")
    bq_d = nc.dram_tensor("bq", [128, 6], f32, kind="ExternalInput")
    bqs_d = nc.dram_tensor("bqs", [128, 6], f32, kind="ExternalInput")
    cs_d = nc.dram_tensor("cs", [2, 128, F4], f32, kind="ExternalInput")
    rpb_d = nc.dram_tensor("rpbt", [H, 2, 128, N], bf16, kind="ExternalInput")
    ident_d = nc.dram_tensor("ident", [128, 128], bf16, kind="ExternalInput")
    projt_d = nc.dram_tensor("projt", [C, C], f32, kind="ExternalInput")
    projb_d = nc.dram_tensor("projb", [1, C], f32, kind="ExternalInput")
    y_d = nc.dram_tensor("y", [n_img, N, C], f32, kind="ExternalOutput")

    from contextlib import ExitStack
    with tile.TileContext(nc) as tc:
        with ExitStack() as stk:
            pool = lambda name, bufs, **kw: stk.enter_context(
                tc.tile_pool(name=name, bufs=bufs, **kw))
            # NOTE: bufs is per-tag. PSUM budget: qkps 2 + vps 1 + psA 1 +
            # psB 1 + aops 1 + sums 1 + yps 1 = 8 banks exactly.
            constp = pool("const", 1)
            xtp = pool("xt", 2)
            qkps = pool("qkps", 2, space="PSUM")
            vps = pool("vps", 1, space="PSUM")
            qkbfp = pool("qkbf", 2)
            ropet = pool("ropet", 1)
            vsbp = pool("vsb", 8)
            scps = pool("scps", 1, space="PSUM")
            probsp = pool("probs", 4)
            aops = pool("aops", 1, space="PSUM")
            sumsp = pool("sums", 1, space="PSUM")
            rsbp = pool("rsb", 2)
            aosbp = pool("aosb", 6)
            rbc = pool("rbc", 2)
            yps = pool("yps", 1, space="PSUM")
            ysbp = pool("ysb", 2)
            otp = pool("otp", 6)

            # ---- constants (batched DMAs) ----
            wt_all = constp.tile([128, 6 * 3 * C], f32r, tag="wtall")
            nc.sync.dma_start(
                wt_all[:].rearrange("p (cc j) -> p cc j", cc=6),
                wt_d.rearrange("(cc p) j -> cc p j", cc=6)
                .transpose((1, 0, 2)).bitcast(f32r))
            wt_sb = [wt_all[:, cc * 3 * C:(cc + 1) * 3 * C] for cc in range(6)]
            pt_all = constp.tile([128, 6 * C], f32r, tag="ptall")
            nc.sync.dma_start(
                pt_all[:].rearrange("p (cc j) -> p cc j", cc=6),
                projt_d.rearrange("(cc p) j -> cc p j", cc=6)
                .transpose((1, 0, 2)).bitcast(f32r))
            projt_sb = [pt_all[:, cc * C:(cc + 1) * C] for cc in range(6)]
            rpb_all = constp.tile([128, H * 2 * N], bf16, tag="rpball")
            nc.sync.dma_start(
                rpb_all[:].rearrange("p (g j) -> p g j", g=H * 2),
                rpb_d.rearrange("h c p j -> (h c) p j").transpose((1, 0, 2)))
            rpb_sb = [(rpb_all[:, (h * 2) * N:(h * 2 + 1) * N],
                       rpb_all[:, (h * 2 + 1) * N:(h * 2 + 2) * N])
                      for h in range(H)]
            projb_bc = constp.tile([128, C], f32, tag="pbbc")
            nc.gpsimd.dma_start(
                projb_bc[:],
                projb_d[:].unsqueeze(1).broadcast_to((1, 128, C)))
            bq_sb = constp.tile([128, 6], f32, tag="bq")
            nc.sync.dma_start(bq_sb[:], bq_d[:])
            bqs_sb = constp.tile([128, 6], f32, tag="bqs")
            nc.sync.dma_start(bqs_sb[:], bqs_d[:])
            cos_sb = constp.tile([128, F4], f32, tag="cos")
            nc.sync.dma_start(cos_sb[:], cs_d[0])
            spm_sb = constp.tile([128, F4], f32, tag="spm")
            nc.sync.dma_start(spm_sb[:], cs_d[1])
            ident_sb = constp.tile([128, 128], bf16, tag="ident")
            nc.sync.dma_start(ident_sb[:], ident_d[:])
            # E-band: column 11 is ones; slicing [:, 11-h:23-h] gives a
            # [128, 12] selector with ones in column h.
            eband = constp.tile([128, 23], bf16, tag="eband")
            nc.vector.memset(eband[:], 0.0)
            nc.vector.memset(eband[:, 11:12], 1.0)

            qk_quad = {}
            v_pairs = {}

            def attention(p, par, xt_ref):
                """Scores/attn/normalize/proj for image pair p (quad slot par)."""
                v_sb = v_pairs.pop(p)
                ao_list = []
                sums_ps = sumsp.tile([12, F2], f32, tag="sums",
                                     padded_shape=[12, 512], name=f"sums{p}")
                # zero the sums bank (sets has_written) so the per-unit sums
                # matmuls can accumulate in any interleaving with start=False
                nc.tensor.matmul(
                    sums_ps[:], lhsT=eband[0:1, 0:12],
                    rhs=rpb_all[0:1, 0:F2], start=True, stop=False,
                    skip_group_check=True)
                for hp in range(6):
                    qro = qk_quad[hp + 100]
                    kro = qk_quad[hp + 6 + 100]
                    ao = aops.tile([128, F2], f32, tag="aops",
                                   padded_shape=[128, 512], name=f"ao{p}{hp}")
                    for ic in range(2):
                        qoff = (par * 2 + ic) * N
                        psA = scps.tile([128, F2], f32, tag="psA",
                                        padded_shape=[128, 512], name=f"psA{p}{hp}{ic}")
                        psB = scps.tile([128, F2], f32, tag="psB",
                                        padded_shape=[128, 512], name=f"psB{p}{hp}{ic}")
                        for ph in range(2):
                            h = 2 * hp + ph
                            cr = ph * N
                            qv = qro[ph * 64:(ph + 1) * 64, qoff:qoff + N]
                            nc.tensor.matmul(
                                psA[:, cr:cr + N], lhsT=ident_sb[:],
                                rhs=rpb_sb[h][0], start=True, stop=False)
                            nc.tensor.matmul(
                                psA[:, cr:cr + N],
                                lhsT=kro[ph * 64:(ph + 1) * 64, qoff:qoff + 128],
                                rhs=qv, start=False, stop=True)
                            nc.tensor.matmul(
                                psB[0:N1, cr:cr + N], lhsT=ident_sb[0:N1, 0:N1],
                                rhs=rpb_sb[h][1][0:N1, :], start=True, stop=False)
                            nc.tensor.matmul(
                                psB[0:N1, cr:cr + N],
                                lhsT=kro[ph * 64:(ph + 1) * 64, qoff + 128:qoff + N],
                                rhs=qv, start=False, stop=True)
                        prA = probsp.tile([128, F2], bf16, tag="prA",
                                          name=f"prA{p}{hp}{ic}")
                        prB = probsp.tile([128, F2], bf16, tag="prB",
                                          name=f"prB{p}{hp}{ic}")
                        nc.scalar.activation(prA[:], psA[:], AF.Exp)
                        nc.scalar.activation(prB[0:N1, :], psB[0:N1, :], AF.Exp)
                        for ph in range(2):
                            h = 2 * hp + ph
                            cr = ph * N
                            nc.tensor.matmul(
                                ao[ph * 64:(ph + 1) * 64, ic * N:(ic + 1) * N],
                                lhsT=v_sb[ic][0][:, h * 64:(h + 1) * 64],
                                rhs=prA[:, cr:cr + N], start=True, stop=False)
                            nc.tensor.matmul(
                                ao[ph * 64:(ph + 1) * 64, ic * N:(ic + 1) * N],
                                lhsT=v_sb[ic][1][0:N1, h * 64:(h + 1) * 64],
                                rhs=prB[0:N1, cr:cr + N], start=False, stop=True)
                            last = (hp == 5 and ic == 1 and ph == 1)
                            nc.tensor.matmul(
                                sums_ps[:, ic * N:(ic + 1) * N],
                                lhsT=eband[:, 11 - h:23 - h],
                                rhs=prA[:, cr:cr + N],
                                start=False, stop=False, skip_group_check=True)
                            nc.tensor.matmul(
                                sums_ps[:, ic * N:(ic + 1) * N],
                                lhsT=eband[0:N1, 11 - h:23 - h],
                                rhs=prB[0:N1, cr:cr + N],
                                start=False, stop=last, skip_group_check=True)
                    aot = aosbp.tile([128, F2], f32, tag="aosb",
                                     name=f"aot{p}{hp}")
                    nc.scalar.activation(aot[:], ao[:], AF.Copy)
                    ao_list.append(aot)

                # ---- normalization: r = exp(-ln(sums)) ----
                lnt = rsbp.tile([12, F2], f32, tag="lnt", name=f"lnt{p}")
                nc.scalar.activation(lnt[:], sums_ps[:], AF.Ln)
                rsm = rsbp.tile([12, F2], f32, tag="rsm", name=f"rsm{p}")
                nc.scalar.activation(rsm[:], lnt[:], AF.Exp, scale=-1.0)
                ot_sb = []
                for hp in range(6):
                    # broadcast r rows across partitions via 0-stride DMA
                    rb = rbc.tile([128, F2], f32, tag="rbc", name=f"rb{p}{hp}")
                    nc.gpsimd.dma_start(
                        rb[0:64, :],
                        rsm[2 * hp:2 * hp + 1, :].unsqueeze(1)
                        .broadcast_to((1, 64, F2)))
                    nc.gpsimd.dma_start(
                        rb[64:128, :],
                        rsm[2 * hp + 1:2 * hp + 2, :].unsqueeze(1)
                        .broadcast_to((1, 64, F2)))
                    ot = otp.tile([128, F2], f32r, tag="ot", name=f"ot{p}{hp}")
                    nc.vector.tensor_mul(ot[:], ao_list[hp][:], rb[:])
                    ot_sb.append(ot)

                # ---- output projection (bias added at eviction) ----
                for ic in range(2):
                    img = (2 * p + ic) % n_img
                    for n_off, n_sz in ((0, N0), (N0, N1)):
                        yt = ysbp.tile([128, C], f32, tag="ysb",
                                       name=f"yt{p}{ic}{n_off}")
                        for ch in range(2):
                            ps = yps.tile([128, 384], f32, tag="yps",
                                          padded_shape=[128, 512],
                                          name=f"yps{p}{ic}{n_off}{ch}")
                            for cc in range(6):
                                nc.tensor.matmul(
                                    ps[0:n_sz, :],
                                    lhsT=ot_sb[cc][:, ic * N + n_off:ic * N + n_off + n_sz],
                                    rhs=projt_sb[cc][:, ch * 384:(ch + 1) * 384],
                                    start=(cc == 0), stop=(cc == 5))
                            nc.vector.tensor_add(
                                yt[0:n_sz, ch * 384:(ch + 1) * 384],
                                ps[0:n_sz, :],
                                projb_bc[0:n_sz, ch * 384:(ch + 1) * 384])
                        nc.sync.dma_start(
                            y_d[img, n_off:n_off + n_sz, :], yt[0:n_sz, :])

            total_pairs = repeat * n_pairs
            for pi in range(total_pairs):
                p = pi % n_pairs
                par = pi % 2
                # ---- load xT for this image pair ----
                xt_sb = []
                for cc in range(6):
                    t = xtp.tile([128, F2], f32r, tag=f"xt{cc}", name=f"xt{pi}{cc}")
                    nc.sync.dma_start(
                        t[:], xt_d[p, cc * 128:(cc + 1) * 128, :].bitcast(f32r))
                    xt_sb.append(t)

                # ---- q/k projection into quad tiles ----
                if par == 0:
                    for m in range(12):
                        qk_quad[m] = qkbfp.tile(
                            [128, F4], bf16, tag=f"qk{m}", name=f"qk{pi}{m}")
                for m in range(12):
                    ps = qkps.tile([128, F2], f32, tag="qkps",
                                   padded_shape=[128, 512], name=f"qkp{pi}{m}")
                    for cc in range(6):
                        nc.tensor.matmul(
                            ps[:],
                            lhsT=wt_sb[cc][:, m * 128:(m + 1) * 128],
                            rhs=xt_sb[cc][:],
                            start=(cc == 0), stop=(cc == 5))
                    dst = qk_quad[m][:, par * F2:(par + 1) * F2]
                    if m < 6:
                        nc.scalar.activation(dst, ps[:], AF.Copy)
                    else:
                        nc.vector.tensor_copy(dst, ps[:])

                # ---- v projection (natural out) ----
                v_sb = []
                for ic in range(2):
                    vts = [vsbp.tile([128, C], bf16, tag="vsb",
                                     name=f"vsb{pi}{ic}{i}") for i in range(2)]
                    for nck, (n_off, n_sz) in enumerate(((0, N0), (N0, N1))):
                        for ch in range(2):
                            ps = vps.tile([128, 384], f32, tag="vps",
                                          padded_shape=[128, 512],
                                          name=f"vps{pi}{ic}{nck}{ch}")
                            for cc in range(6):
                                nc.tensor.matmul(
                                    ps[0:n_sz, :],
                                    lhsT=xt_sb[cc][:, ic * N + n_off:ic * N + n_off + n_sz],
                                    rhs=wt_sb[cc][:, 2 * C + ch * 384:2 * C + (ch + 1) * 384],
                                    start=(cc == 0), stop=(cc == 5))
                            nc.scalar.activation(
                                vts[nck][0:n_sz, ch * 384:(ch + 1) * 384],
                                ps[0:n_sz, :], AF.Copy)
                    v_sb.append(vts)
                v_pairs[p] = v_sb

                # ---- rope on the full quad, then attention for both pairs ----
                if par == 1 or pi == total_pairs - 1:
                    fw = F4 if par == 1 else F2
                    for m in range(12):
                        src = qk_quad[m]
                        qs = ropet.tile([128, F4], bf16, tag="qs", name=f"qs{pi}{m}")
                        nc.vector.stream_shuffle(qs[:, 0:fw], src[:, 0:fw], SHUF_MASK)
                        u = ropet.tile([128, F4], bf16, tag="u", name=f"u{pi}{m}")
                        v = ropet.tile([128, F4], bf16, tag="v", name=f"v{pi}{m}")
                        if m < 6:
                            nc.vector.scalar_tensor_tensor(
                                out=u[:, 0:fw], in0=src[:, 0:fw],
                                scalar=bq_sb[:, m:m + 1],
                                in1=cos_sb[:, 0:fw], op0=OP.add, op1=OP.mult)
                            nc.vector.scalar_tensor_tensor(
                                out=v[:, 0:fw], in0=qs[:, 0:fw],
                                scalar=bqs_sb[:, m:m + 1],
                                in1=spm_sb[:, 0:fw], op0=OP.add, op1=OP.mult)
                        else:
                            nc.vector.tensor_mul(u[:, 0:fw], src[:, 0:fw],
                                                 cos_sb[:, 0:fw])
                            nc.vector.tensor_mul(v[:, 0:fw], qs[:, 0:fw],
                                                 spm_sb[:, 0:fw])
                        nc.vector.tensor_add(src[:, 0:fw], u[:, 0:fw], v[:, 0:fw])
                        qk_quad[m + 100] = src
                    if par == 1:
                        attention((pi - 1) % n_pairs, 0, None)
                    attention(p, par, None)
    nc.compile()
    return nc


def host_prepare(inputs):
    x = np.asarray(inputs["x"], np.float32)
    qkv_w = np.asarray(inputs["qkv_w"], np.float32)
    scale = D ** -0.5
    Wq = qkv_w[:C] + np.asarray(inputs["lora_q_b"]) @ np.asarray(inputs["lora_q_a"])
    Wk = qkv_w[C:2 * C] + np.asarray(inputs["lora_k_b"]) @ np.asarray(inputs["lora_k_a"])
    Wv = qkv_w[2 * C:] + np.asarray(inputs["lora_v_b"]) @ np.asarray(inputs["lora_v_a"])
    p64 = _perm64()
    perm = (np.arange(H)[:, None] * D + p64[None, :]).ravel()
    Wq_de = (Wq * scale)[perm]
    bq_de = (np.asarray(inputs["q_bias"], np.float32) * scale)[perm]
    Wk_de = Wk[perm]
    wt = np.ascontiguousarray(np.concatenate([Wq_de, Wk_de, Wv], 0).T)

    bq = np.ascontiguousarray(bq_de.reshape(6, 128).T)
    bqs = np.ascontiguousarray(
        np.stack([_swap16_rows(bq_de[i * 128:(i + 1) * 128]) for i in range(6)], 1))

    cos_f = np.ones((N, D), np.float32)
    cos_f[1:] = np.asarray(inputs["rope_cos"], np.float32)
    sin_f = np.zeros((N, D), np.float32)
    sin_f[1:] = np.asarray(inputs["rope_sin"], np.float32)
    cos_de = np.ascontiguousarray(cos_f[:, p64].T)
    spm = np.ascontiguousarray(sin_f[:, p64].T)
    for blk in range(2):
        spm[blk * 32:blk * 32 + 16] *= -1.0
    cs = np.stack([
        np.tile(np.vstack([cos_de, cos_de]), (1, 4)),
        np.tile(np.vstack([spm, spm]), (1, 4)),
    ]).astype(np.float32)

    rel_table = np.asarray(inputs["rel_table"], np.float32)
    rel_index = np.asarray(inputs["rel_index"])
    rpb = rel_table[rel_index.reshape(-1)].reshape(N, N, H)
    rpbT = rpb.transpose(2, 1, 0)  # [h, j, i]
    rpbt = np.zeros((H, 2, 128, N), ml_dtypes.bfloat16)
    rpbt[:, 0] = rpbT[:, 0:128, :].astype(ml_dtypes.bfloat16)
    rpbt[:, 1, 0:N1] = rpbT[:, 128:N, :].astype(ml_dtypes.bfloat16)

    ident = np.eye(128, dtype=ml_dtypes.bfloat16)
    proj_w = np.asarray(inputs["proj_w"], np.float32)
    projt = np.ascontiguousarray(proj_w.T)
    projb = (np.asarray(inputs["proj_b"], np.float32)
             + proj_w @ np.asarray(inputs["v_bias"], np.float32)).reshape(1, C)

    xt = x.transpose(0, 2, 1)  # [B, C, N]
    xt_pairs = np.ascontiguousarray(
        xt.reshape(B // 2, 2, C, N).transpose(0, 2, 1, 3).reshape(B // 2, C, 2 * N))

    shared = dict(wt=wt, bq=bq, bqs=bqs, cs=cs, rpbt=rpbt, ident=ident,
                  projt=projt, projb=projb)
    per_core = []
    ppc = BPC // 2
    for c in range(NCORES):
        m = dict(shared)
        m["xt"] = np.ascontiguousarray(xt_pairs[c * ppc:(c + 1) * ppc])
        per_core.append(m)
    return per_core


def kernel(**inputs):
    from concourse.bass_utils import run_bass_kernel_spmd
    in_maps = host_prepare(inputs)
    if "nc" not in _cache:
        _cache["nc"] = build_program()
    nc = _cache["nc"]
    res = run_bass_kernel_spmd(nc, in_maps, list(range(NCORES))).results
    y = np.concatenate([res[c]["y"] for c in range(NCORES)], 0)
    return np.ascontiguousarray(y.astype(np.float32))
